# revision 14
# baseline (speedup 1.0000x reference)
import numpy as np
import jax
import jax.numpy as jnp
from ml_dtypes import bfloat16

B = 8192        # graphs
NPG = 39        # nodes per graph
N = B * NPG
NC = 8          # neuron cores
GPC = B // NC   # real graphs per core
GPAD = 1026     # padded graphs per core (multiple of 3)
PK = GPAD // 3  # packs of 3 graphs -> 117-node block-diag tiles
NEG = 0.2

_BF = jnp.bfloat16
_F32 = jnp.float32


def _dot(a, b):
    return jax.lax.dot_general(
        a.astype(_BF), b.astype(_BF),
        (((a.ndim - 1,), (0,)), ((), ())),
        preferred_element_type=_F32)


def _gat(h_in, A, W, a_s, a_d, b):
    # h_in [GPAD,39,fi]; A [PK,117,117] bf16 block-diag edge counts
    # (incl self loops; zeros off-diagonal mask cross-graph pairs).
    # Attention logits are bounded (|E|<5 for this data) so softmax
    # needs no max-subtraction; the denominator rides along the
    # aggregation matmul as an appended ones-column.
    fi = h_in.shape[2]
    fo = W.shape[1]
    hr = h_in.reshape(GPAD * NPG, fi)
    H = _dot(hr, W)                                  # [GPAD*39, fo] f32
    h = H.astype(_BF).reshape(PK, 117, fo)
    # logits via an output-transposed dot: rows come out contiguous,
    # so the [PK,117,1]/[PK,1,117] views below are free reshapes
    Wsd = jnp.stack([W @ a_s, W @ a_d], axis=1)      # [fi, 2]
    sdT = jax.lax.dot_general(
        Wsd.astype(_BF), hr.astype(_BF), (((0,), (1,)), ((), ())),
        preferred_element_type=_F32)                 # [2, GPAD*39]
    # outer-sum via batched K=2 matmul: z[p] = [s|1] @ [[1],[d]]
    one = jnp.ones((1, GPAD * NPG), _BF)
    sd1 = jnp.concatenate([sdT.astype(_BF), one], 0)     # [3, GPAD*39]
    zl = sd1[jnp.array([0, 2])].T.reshape(PK, 117, 2)    # [s | ones]
    zr = sd1[jnp.array([2, 1])].reshape(2, PK, 117).transpose(1, 0, 2)
    z = jax.lax.dot_general(
        zl, zr, (((2,), (1,)), ((0,), (0,))),
        preferred_element_type=_F32)                     # [PK,117(s),117(d)]
    E = jnp.maximum(z, NEG * z)                      # leaky relu, slope<1
    w = jnp.exp(E + A)                               # A = log-counts; masked
    #   entries hold -30 -> exp ~1e-13, negligible vs real weights
    hw = jnp.concatenate(
        [h, jnp.ones((PK, 117, 1), _BF)], axis=2)    # [PK,117(s),fo+1]
    agg = jax.lax.dot_general(
        w, hw, (((1,), (1,)), ((0,), (0,))),
        preferred_element_type=_F32)                 # [PK,117(d),fo+1]
    out = agg[..., :fo] / agg[..., fo:]              # denom>0: self-loops
    return jax.nn.relu(out + b).reshape(GPAD, NPG, fo)


def _fwd(x, A, params):
    (W1, as1, ad1, b1, W2, as2, ad2, b2, W3, as3, ad3, b3,
     W4, as4, ad4, b4, lw1, lb1, lw2, lb2, lw3, lb3) = params
    h1 = _gat(x[..., None], A, W1, as1, ad1, b1)
    h2 = _gat(h1, A, W2, as2, ad2, b2)
    h3 = _gat(h2, A, W3, as3, ad3, b3)
    h4 = _gat(h3, A, W4, as4, ad4, b4)
    f = jnp.concatenate([
        x, h1.reshape(GPAD, -1), h2.reshape(GPAD, -1),
        h3.reshape(GPAD, -1), h4.reshape(GPAD, -1),
        jnp.max(x, axis=1, keepdims=True),
        jnp.max(h1, axis=1), jnp.max(h2, axis=1),
        jnp.max(h3, axis=1), jnp.max(h4, axis=1)], axis=1)   # [GPAD,4560]
    f = jax.nn.relu(_dot(f, lw1) + lb1)
    f = jax.nn.relu(_dot(f, lw2) + lb2)
    return _dot(f, lw3) + lb3


_pmapped = jax.pmap(_fwd, in_axes=(0, 0, None))


def _build_A(edge_index):
    # block-diag packed counts: [NC, PK, 117, 117] bf16, 3 graphs/pack
    src = np.asarray(edge_index[0], dtype=np.int64)
    dst = np.asarray(edge_index[1], dtype=np.int64)
    g = dst // NPG
    core = g // GPC
    gl = g - core * GPC
    p = core * PK + gl // 3
    a = gl - (gl // 3) * 3
    dl = dst - g * NPG
    sl = src - g * NPG
    idx = (p * 117 + a * NPG + sl) * 117 + (a * NPG + dl)
    A = np.bincount(idx, minlength=NC * PK * 117 * 117)
    A = A.astype(np.float32).reshape(NC, PK, 117, 117)
    di = np.arange(117)
    A[:, :, di, di] += 1.0   # self loops (pad graphs become identity)
    with np.errstate(divide='ignore'):
        L = np.log(A)        # fold count-multiply into the exponent
    L[A == 0.0] = -30.0      # finite mask: exp(-30) ~ 9e-14
    return L.astype(bfloat16)


def _prep(inputs):
    x = np.asarray(inputs['x'], np.float32).reshape(NC, GPC, NPG)
    xp = np.zeros((NC, GPAD, NPG), np.float32)
    xp[:, :GPC] = x
    A = _build_A(inputs['edge_index'])
    pnames = []
    for li in range(1, 5):
        pnames += [f'W{li}', f'as{li}', f'ad{li}', f'b{li}']
    pnames += ['lw1', 'lb1', 'lw2', 'lb2', 'lw3', 'lb3']
    params = tuple(np.asarray(inputs[k], np.float32) for k in pnames)
    return xp, A, params


def kernel(**inputs):
    xp, A, params = _prep(inputs)
    out = _pmapped(xp, A, tuple(jnp.asarray(p) for p in params))
    return np.asarray(out[:, :GPC]).reshape(B, 9).astype(np.float32)


# revision 18
# speedup vs baseline: 1.1583x; 1.1583x over previous
import numpy as np
import jax
import jax.numpy as jnp
from ml_dtypes import bfloat16

B = 8192        # graphs
NPG = 39        # nodes per graph
N = B * NPG
NC = 8          # neuron cores
GPC = B // NC   # real graphs per core
GPAD = 1026     # padded graphs per core (multiple of 3)
PK = GPAD // 3  # packs of 3 graphs -> 117-node block-diag tiles
NEG = 0.2

_BF = jnp.bfloat16
_F32 = jnp.float32


def _dot(a, b, out_dt=_F32):
    return jax.lax.dot_general(
        a.astype(_BF), b.astype(_BF),
        (((a.ndim - 1,), (0,)), ((), ())),
        preferred_element_type=out_dt)


def _gat(h_in, A, W, a_s, a_d, b):
    # h_in [GPAD,39,fi]; A [PK,117,117] bf16 block-diag edge counts
    # (incl self loops; zeros off-diagonal mask cross-graph pairs).
    # Attention logits are bounded (|E|<5 for this data) so softmax
    # needs no max-subtraction; the denominator rides along the
    # aggregation matmul as an appended ones-column.
    fi = h_in.shape[2]
    fo = W.shape[1]
    hr = h_in.reshape(GPAD * NPG, fi)
    h = _dot(hr, W, _BF).reshape(PK, 117, fo)        # bf16 direct: its only
    #   consumer is the bf16 aggregation matmul, so skip the f32 roundtrip
    # logits via an output-transposed dot: rows come out contiguous,
    # so the [PK,117,1]/[PK,1,117] views below are free reshapes
    Wsd = jnp.stack([W @ a_s, W @ a_d], axis=1)      # [fi, 2]
    sdT = jax.lax.dot_general(
        Wsd.astype(_BF), hr.astype(_BF), (((0,), (1,)), ((), ())),
        preferred_element_type=_F32)                 # [2, GPAD*39]
    s = sdT[0].astype(_BF).reshape(PK, 117, 1)       # source logit col
    d = sdT[1].astype(_BF).reshape(PK, 1, 117)       # dest logit row
    z = s + d                                        # [PK,117(s),117(d)]
    E = jnp.maximum(z, NEG * z)                      # leaky relu, slope<1
    w = jnp.exp(E + A)                               # A = log-counts; masked
    #   entries hold -30 -> exp ~1e-13, negligible vs real weights
    hw = jnp.concatenate(
        [h, jnp.ones((PK, 117, 1), _BF)], axis=2)    # [PK,117(s),fo+1]
    agg = jax.lax.dot_general(
        w, hw, (((1,), (1,)), ((0,), (0,))),
        preferred_element_type=_F32)                 # [PK,117(d),fo+1]
    out = agg[..., :fo] / agg[..., fo:]              # denom>0: self-loops
    return jax.nn.relu(out + b).reshape(GPAD, NPG, fo)


def _fwd(x, A, params):
    (W1, as1, ad1, b1, W2, as2, ad2, b2, W3, as3, ad3, b3,
     W4, as4, ad4, b4, lw1, lb1, lw2, lb2, lw3, lb3) = params
    h1 = _gat(x[..., None], A, W1, as1, ad1, b1)
    h2 = _gat(h1, A, W2, as2, ad2, b2)
    h3 = _gat(h2, A, W3, as3, ad3, b3)
    h4 = _gat(h3, A, W4, as4, ad4, b4)
    f = jnp.concatenate([
        x, h1.reshape(GPAD, -1), h2.reshape(GPAD, -1),
        h3.reshape(GPAD, -1), h4.reshape(GPAD, -1),
        jnp.max(x, axis=1, keepdims=True),
        jnp.max(h1, axis=1), jnp.max(h2, axis=1),
        jnp.max(h3, axis=1), jnp.max(h4, axis=1)], axis=1)   # [GPAD,4560]
    f = jax.nn.relu(_dot(f, lw1, _BF) + lb1.astype(_BF))
    f = jax.nn.relu(_dot(f, lw2, _BF) + lb2.astype(_BF))
    return _dot(f, lw3) + lb3


_pmapped = jax.pmap(_fwd, in_axes=(0, 0, None))


def _build_A(edge_index):
    # block-diag packed counts: [NC, PK, 117, 117] bf16, 3 graphs/pack
    src = np.asarray(edge_index[0], dtype=np.int64)
    dst = np.asarray(edge_index[1], dtype=np.int64)
    g = dst // NPG
    core = g // GPC
    gl = g - core * GPC
    p = core * PK + gl // 3
    a = gl - (gl // 3) * 3
    dl = dst - g * NPG
    sl = src - g * NPG
    idx = (p * 117 + a * NPG + sl) * 117 + (a * NPG + dl)
    A = np.bincount(idx, minlength=NC * PK * 117 * 117)
    A = A.astype(np.float32).reshape(NC, PK, 117, 117)
    di = np.arange(117)
    A[:, :, di, di] += 1.0   # self loops (pad graphs become identity)
    with np.errstate(divide='ignore'):
        L = np.log(A)        # fold count-multiply into the exponent
    L[A == 0.0] = -30.0      # finite mask: exp(-30) ~ 9e-14
    return L.astype(bfloat16)


def _prep(inputs):
    x = np.asarray(inputs['x'], np.float32).reshape(NC, GPC, NPG)
    xp = np.zeros((NC, GPAD, NPG), np.float32)
    xp[:, :GPC] = x
    A = _build_A(inputs['edge_index'])
    pnames = []
    for li in range(1, 5):
        pnames += [f'W{li}', f'as{li}', f'ad{li}', f'b{li}']
    pnames += ['lw1', 'lb1', 'lw2', 'lb2', 'lw3', 'lb3']
    params = tuple(np.asarray(inputs[k], np.float32) for k in pnames)
    return xp, A, params


def kernel(**inputs):
    xp, A, params = _prep(inputs)
    out = _pmapped(xp, A, tuple(jnp.asarray(p) for p in params))
    return np.asarray(out[:, :GPC]).reshape(B, 9).astype(np.float32)
